# revision 11
# baseline (speedup 1.0000x reference)
"""Cross-attention with LoRA (Q and V adapters) on 8 TRN2 NeuronCores.

Sharding: core = (b, g) where b = batch index (2), g = head group (4 groups
of 4 heads).  Data parallel over batch, tensor parallel over heads for the
QKV projections; the output projection is column-sharded so each core
produces a partial (1024, 2048) output that the host sums per batch.

All device tensors are pre-transposed on the host so the kernel needs no
on-chip transposes:
  xt   = x[b].T               (1024, 2048)   [embed, seq]
  ctxt = context[b].T         (1024, 2048)   [embed, ctx]
  wqT  = (Wq[g]/8).T          (1024, 256)    1/sqrt(hd) folded in
  bqT  = (Bq[g]/(128*8)).T    (128, 256)     LoRA 1/r and 1/sqrt(hd) folded
  wkT  = Wk[g].T              (1024, 256)
  wvT  = Wv[g].T              (1024, 256)
  aqT/avT = Aq.T/Av.T         (1024, 128)    replicated
  bvT  = (Bv[g]/128).T        (128, 256)
  woT  = Wo[:, g].T           (256, 1024)
Output out_t = (x-partial of out).T per core; host computes
  out[b] = sum_g out_t[(b,g)].T
"""

import numpy as np

import concourse.bass as bass
import concourse.tile as tile
from concourse import bacc, mybir
from concourse.bass import ts
from concourse.bass_utils import run_bass_kernel_spmd

F32 = mybir.dt.float32
BF16 = mybir.dt.bfloat16
EXP = mybir.ActivationFunctionType.Exp

P = 128          # partitions
D = 1024         # embed dim
KO = D // P      # embed chunks (8)
HG = 4           # heads per core
HD = 64          # head dim
GD = HG * HD     # group dim (256)
R = 128          # LoRA rank
NMM = 512        # matmul moving-dim chunk
AQ = 512         # activation streaming quarter (phase-1 seq chunk)
SQB = 1024       # phase-2 query block


def build_nc(S=2048, C=2048):
    """Build + compile the per-core Bass program (identical on all cores)."""
    nc = bacc.Bacc("TRN2", target_bir_lowering=False, debug=False)

    xt = nc.dram_tensor("xt", [D, S], BF16, kind="ExternalInput").ap()
    ctxt = nc.dram_tensor("ctxt", [D, C], BF16, kind="ExternalInput").ap()
    wqT = nc.dram_tensor("wqT", [D, GD], BF16, kind="ExternalInput").ap()
    aqT = nc.dram_tensor("aqT", [D, R], BF16, kind="ExternalInput").ap()
    bqT = nc.dram_tensor("bqT", [R, GD], BF16, kind="ExternalInput").ap()
    wkT = nc.dram_tensor("wkT", [D, GD], BF16, kind="ExternalInput").ap()
    wvT = nc.dram_tensor("wvT", [D, GD], BF16, kind="ExternalInput").ap()
    avT = nc.dram_tensor("avT", [D, R], BF16, kind="ExternalInput").ap()
    bvT = nc.dram_tensor("bvT", [R, GD], BF16, kind="ExternalInput").ap()
    woT = nc.dram_tensor("woT", [GD, D], BF16, kind="ExternalInput").ap()
    out_t = nc.dram_tensor("out_t", [D, S], F32, kind="ExternalOutput").ap()

    with tile.TileContext(nc) as tc:
        _build(tc, xt, ctxt, wqT, aqT, bqT, wkT, wvT, avT, bvT, woT, out_t, S, C)
    nc.compile()
    return nc


def _build(tc, xt, ctxt, wqT, aqT, bqT, wkT, wvT, avT, bvT, woT, out_t, S, C):
    nc = tc.nc
    CK = C // P      # context seq chunks (16)
    sqb = min(SQB, S)  # phase-2 query block

    xt_r = xt.rearrange("(ko p) s -> p ko s", p=P)
    ctxt_r = ctxt.rearrange("(ko p) s -> p ko s", p=P)
    out_r = out_t.rearrange("(ko p) s -> ko p s", p=P)

    with (
        tc.tile_pool(name="w", bufs=1) as wpool,
        tc.tile_pool(name="wbig", bufs=2) as wbig,
        tc.tile_pool(name="acts", bufs=2) as actsp,
        tc.tile_pool(name="kqv", bufs=1) as kqv,
        tc.tile_pool(name="lora", bufs=1) as lorap,
        tc.tile_pool(name="pt", bufs=4) as ptp,
        tc.tile_pool(name="small", bufs=2) as smallp,
        tc.tile_pool(name="outsb", bufs=2) as outp,
    ):
        # ---- small weights (resident) ----
        aq_sb = wpool.tile([P, KO, R], BF16, tag="aq")
        nc.sync.dma_start(aq_sb[:], aqT.rearrange("(ko p) r -> p ko r", p=P))
        av_sb = wpool.tile([P, KO, R], BF16, tag="av")
        nc.sync.dma_start(av_sb[:], avT.rearrange("(ko p) r -> p ko r", p=P))
        bq_sb = wpool.tile([R, GD], BF16, tag="bq")
        nc.sync.dma_start(bq_sb[:], bqT)
        bv_sb = wpool.tile([R, GD], BF16, tag="bv")
        nc.sync.dma_start(bv_sb[:], bvT)

        # ---- big weights cycle through 2 slots: wk, wv, wq, wo ----
        wk_sb = wbig.tile([P, KO, GD], BF16, tag="wbig")
        nc.sync.dma_start(wk_sb[:], wkT.rearrange("(ko p) m -> p ko m", p=P))
        wv_sb = wbig.tile([P, KO, GD], BF16, tag="wbig")
        nc.sync.dma_start(wv_sb[:], wvT.rearrange("(ko p) m -> p ko m", p=P))

        # ---- persistent activations ----
        kt_sb = kqv.tile([P, 2, C], BF16, tag="kt")       # K^T  [hd, ctx]
        qt_sb = kqv.tile([P, 2, S], BF16, tag="qt")       # Q^T  [hd, seq]
        vaug_sb = kqv.tile([P, CK, HG, HD + 1], BF16, tag="vaug")  # V + ones col
        att_sb = kqv.tile([P, 2, S], BF16, tag="att")     # attn out^T (normalized)
        tv_sb = lorap.tile([R, C], BF16, tag="tv")
        tq_sb = lorap.tile([R, S], BF16, tag="tq")

        nc.vector.memset(vaug_sb[:, :, :, HD], 1.0)

        # ================= phase 1a: context -> Kt, V =================
        with (
            tc.tile_pool(name="psum1", bufs=4, space="PSUM") as psum1,
            tc.tile_pool(name="psumv", bufs=2, space="PSUM") as psumv,
        ):
            for q in range(C // AQ):
                sl = slice(q * AQ, (q + 1) * AQ)
                ctx_sb = actsp.tile([P, KO, AQ], BF16, tag="acts")
                nc.sync.dma_start(ctx_sb[:], ctxt_r[:, :, sl])

                # tv = Av @ ctx^T  -> [R, ctx-quarter]
                tvp = psum1.tile([P, NMM], F32, tag="proj")
                for k in range(KO):
                    nc.tensor.matmul(
                        tvp[:], (av_sb[:, k, :]), (ctx_sb[:, k, :]),
                        start=(k == 0), stop=(k == KO - 1),
                    )
                nc.vector.tensor_copy(tv_sb[:, sl], tvp[:])

                # Kt quarter
                for m in range(2):
                    kp = psum1.tile([P, NMM], F32, tag="proj")
                    for k in range(KO):
                        nc.tensor.matmul(
                            kp[:], (wk_sb[:, k, ts(m, P)]), (ctx_sb[:, k, :]),
                            start=(k == 0), stop=(k == KO - 1),
                        )
                    nc.vector.tensor_copy(kt_sb[:, m, sl], kp[:])

                # V quarter (normal layout, head-interleaved with ones col)
                for mloc in range(AQ // P):
                    vp = psumv.tile([P, GD], F32, tag="vproj")
                    for k in range(KO):
                        nc.tensor.matmul(
                            vp[:], (ctx_sb[:, k, ts(mloc, P)]), (wv_sb[:, k, :]),
                            start=(k == 0), stop=False,
                        )
                    nc.tensor.matmul(
                        vp[:], (tv_sb[:, q * AQ + mloc * P:q * AQ + (mloc + 1) * P]),
                        (bv_sb[:]), start=False, stop=True,
                    )
                    mg = q * (AQ // P) + mloc
                    nc.vector.tensor_copy(
                        vaug_sb[:, mg, :, 0:HD],
                        vp[:].rearrange("p (h d) -> p h d", h=HG),
                    )

            # ================= phase 1b: x -> Qt =================
            wq_sb = wbig.tile([P, KO, GD], BF16, tag="wbig")
            nc.sync.dma_start(wq_sb[:], wqT.rearrange("(ko p) m -> p ko m", p=P))

            for q in range(S // AQ):
                sl = slice(q * AQ, (q + 1) * AQ)
                x_sb = actsp.tile([P, KO, AQ], BF16, tag="acts")
                nc.sync.dma_start(x_sb[:], xt_r[:, :, sl])

                tqp = psum1.tile([P, NMM], F32, tag="proj")
                for k in range(KO):
                    nc.tensor.matmul(
                        tqp[:], (aq_sb[:, k, :]), (x_sb[:, k, :]),
                        start=(k == 0), stop=(k == KO - 1),
                    )
                nc.vector.tensor_copy(tq_sb[:, sl], tqp[:])

                for m in range(2):
                    qp = psum1.tile([P, NMM], F32, tag="proj")
                    for k in range(KO):
                        nc.tensor.matmul(
                            qp[:], (wq_sb[:, k, ts(m, P)]), (x_sb[:, k, :]),
                            start=(k == 0), stop=False,
                        )
                    nc.tensor.matmul(
                        qp[:], (bq_sb[:, ts(m, P)]), (tq_sb[:, sl]),
                        start=False, stop=True,
                    )
                    nc.vector.tensor_copy(qt_sb[:, m, sl], qp[:])

        # ================= phase 2: attention =================
        wo_sb = wbig.tile([P, 2, D], BF16, tag="wbig")
        nc.sync.dma_start(wo_sb[:], woT.rearrange("(j p) d -> p j d", p=P))

        with (
            tc.tile_pool(name="st", bufs=2, space="PSUM") as stp,
            tc.tile_pool(name="ot", bufs=2, space="PSUM") as otp,
        ):
            for qb in range(S // sqb):
                for h in range(HG):
                    hp = (h % 2) * HD
                    hc = h // 2
                    ot = otp.tile([HD + 1, sqb], F32, tag="ot")

                    def attn_v(sk, pt):
                        for n in range(sqb // NMM):
                            nc.tensor.matmul(
                                ot[:, ts(n, NMM)],
                                (vaug_sb[:, sk, h, :]),
                                (pt[:, ts(n, NMM)]),
                                start=(sk == 0), stop=(sk == CK - 1),
                            )

                    # software-pipelined: attnV for iteration sk-1 is emitted
                    # after scores/exp of iteration sk, so the PE stream never
                    # head-of-line blocks on the current iteration's ACT.
                    prev = None
                    for sk in range(CK):
                        st = stp.tile([P, sqb], F32, tag="st")
                        for n in range(sqb // NMM):
                            nc.tensor.matmul(
                                st[:, ts(n, NMM)],
                                (kt_sb[hp:hp + HD, hc, ts(sk, P)]),
                                (qt_sb[hp:hp + HD, hc,
                                         qb * sqb + n * NMM:qb * sqb + (n + 1) * NMM]),
                                start=True, stop=True,
                            )
                        pt = ptp.tile([P, sqb], BF16, tag="pt")
                        nc.scalar.activation(pt[:], st[:], EXP)
                        if prev is not None:
                            attn_v(*prev)
                        prev = (sk, pt)
                    attn_v(*prev)
                    # normalize: rows 0..63 are O^T, row 64 is the exp rowsum
                    rr = smallp.tile([1, sqb], F32, tag="rr")
                    nc.vector.reciprocal(rr[:], ot[HD:HD + 1, :])
                    rb = smallp.tile([HD, sqb], F32, tag="rb")
                    nc.gpsimd.partition_broadcast(rb[:], rr[:])
                    nc.vector.tensor_mul(
                        att_sb[hp:hp + HD, hc,
                               qb * sqb:(qb + 1) * sqb],
                        ot[0:HD, :], rb[:],
                    )

                # ---- out-projection for this query block (PSUM via st tag) ----
                for e in range(KO):
                    osb = outp.tile([P, sqb], F32, tag="osb")
                    for n in range(sqb // NMM):
                        ng = qb * (sqb // NMM) + n
                        op = stp.tile([P, NMM], F32, tag="st")
                        for j in range(2):
                            nc.tensor.matmul(
                                op[:], (wo_sb[:, j, ts(e, P)]),
                                (att_sb[:, j, ts(ng, NMM)]),
                                start=(j == 0), stop=(j == 1),
                            )
                        nc.vector.tensor_copy(osb[:, ts(n, NMM)], op[:])
                    nc.sync.dma_start(out_r[e][:, qb * sqb:(qb + 1) * sqb], osb[:])


# ---------------------------------------------------------------------------
# Host side
# ---------------------------------------------------------------------------

_NC_CACHE = {}


def _get_nc(S=2048, C=2048):
    key = (S, C)
    if key not in _NC_CACHE:
        _NC_CACHE[key] = build_nc(S, C)
    return _NC_CACHE[key]


def shard_inputs(x, context, Wq, Aq, Bq, Wk, Wv, Av, Bv, Wo):
    """Build the 8 per-core input maps (host-side shard + transpose + scale +
    bf16 cast)."""
    import ml_dtypes

    bf16 = ml_dtypes.bfloat16
    f = lambda a: np.ascontiguousarray(np.asarray(a, dtype=np.float32))
    c = lambda a: np.ascontiguousarray(a).astype(bf16)
    x, context = f(x), f(context)
    Wq, Aq, Bq, Wk, Wv, Av, Bv, Wo = map(f, (Wq, Aq, Bq, Wk, Wv, Av, Bv, Wo))
    sd = 8.0  # sqrt(head_dim)
    lr = 128.0  # LoRA rank (scale = 1/r)
    aqT = c(Aq.T)
    avT = c(Av.T)
    in_maps = []
    for core in range(8):
        b, g = core // 4, core % 4
        sl = slice(g * GD, (g + 1) * GD)
        in_maps.append({
            "xt": c(x[b].T),
            "ctxt": c(context[b].T),
            "wqT": c(Wq[sl].T / sd),
            "aqT": aqT,
            "bqT": c(Bq[sl].T / (lr * sd)),
            "wkT": c(Wk[sl].T),
            "wvT": c(Wv[sl].T),
            "avT": avT,
            "bvT": c(Bv[sl].T / lr),
            "woT": c(Wo[:, sl].T),
        })
    return in_maps


def unshard_output(results, B=2, S=2048):
    out = np.zeros((B, S, D), np.float32)
    for core in range(8):
        b = core // 4
        out[b] += results[core]["out_t"].T
    return out


def kernel(x, context, Wq, Aq, Bq, Wk, Wv, Av, Bv, Wo, _trace=False):
    nc = _get_nc()
    in_maps = shard_inputs(x, context, Wq, Aq, Bq, Wk, Wv, Av, Bv, Wo)
    res = run_bass_kernel_spmd(nc, in_maps, core_ids=list(range(8)), trace=_trace)
    out = unshard_output(res.results)
    if _trace:
        kernel.last_result = res
    return out


# revision 15
# speedup vs baseline: 1.3741x; 1.3741x over previous
"""Cross-attention with LoRA (Q and V adapters) on 8 TRN2 NeuronCores.

Sharding: core = (b, g) where b = batch index (2), g = head group (4 groups
of 4 heads).  Data parallel over batch, tensor parallel over heads for the
QKV projections; the output projection is column-sharded so each core
produces a partial (1024, 2048) output that the host sums per batch.

All device tensors are pre-transposed on the host so the kernel needs no
on-chip transposes:
  xt   = x[b].T               (1024, 2048)   [embed, seq]
  ctxt = context[b].T         (1024, 2048)   [embed, ctx]
  wqT  = (Wq[g]/8).T          (1024, 256)    1/sqrt(hd) folded in
  bqT  = (Bq[g]/(128*8)).T    (128, 256)     LoRA 1/r and 1/sqrt(hd) folded
  wkT  = Wk[g].T              (1024, 256)
  wvT  = Wv[g].T              (1024, 256)
  aqT/avT = Aq.T/Av.T         (1024, 128)    replicated
  bvT  = (Bv[g]/128).T        (128, 256)
  woT  = Wo[:, g].T           (256, 1024)
Output out_t = (x-partial of out).T per core; host computes
  out[b] = sum_g out_t[(b,g)].T
"""

import numpy as np

import concourse.bass as bass
import concourse.tile as tile
from concourse import bacc, mybir
from concourse.bass import ts
from concourse.bass_utils import run_bass_kernel_spmd

F32 = mybir.dt.float32
BF16 = mybir.dt.bfloat16
EXP = mybir.ActivationFunctionType.Exp

P = 128          # partitions
D = 1024         # embed dim
KO = D // P      # embed chunks (8)
HG = 4           # heads per core
HD = 64          # head dim
GD = HG * HD     # group dim (256)
R = 128          # LoRA rank
NMM = 512        # matmul moving-dim chunk
AQ = 512         # activation streaming quarter (phase-1 seq chunk)
SQB = 1024       # phase-2 query block


def build_nc(S=2048, C=2048):
    """Build + compile the per-core Bass program (identical on all cores)."""
    nc = bacc.Bacc("TRN2", target_bir_lowering=False, debug=False)

    xt = nc.dram_tensor("xt", [D, S], BF16, kind="ExternalInput").ap()
    ctxt = nc.dram_tensor("ctxt", [D, C], BF16, kind="ExternalInput").ap()
    wqT = nc.dram_tensor("wqT", [D, GD], BF16, kind="ExternalInput").ap()
    aqT = nc.dram_tensor("aqT", [D, R], BF16, kind="ExternalInput").ap()
    bqT = nc.dram_tensor("bqT", [R, GD], BF16, kind="ExternalInput").ap()
    wkT = nc.dram_tensor("wkT", [D, GD], BF16, kind="ExternalInput").ap()
    wvT = nc.dram_tensor("wvT", [D, GD], BF16, kind="ExternalInput").ap()
    avT = nc.dram_tensor("avT", [D, R], BF16, kind="ExternalInput").ap()
    bvT = nc.dram_tensor("bvT", [R, GD], BF16, kind="ExternalInput").ap()
    woT = nc.dram_tensor("woT", [GD, D], BF16, kind="ExternalInput").ap()
    out_t = nc.dram_tensor("out_t", [D, S], F32, kind="ExternalOutput").ap()

    with tile.TileContext(nc) as tc:
        _build(tc, xt, ctxt, wqT, aqT, bqT, wkT, wvT, avT, bvT, woT, out_t, S, C)
    nc.compile()
    return nc


def _build(tc, xt, ctxt, wqT, aqT, bqT, wkT, wvT, avT, bvT, woT, out_t, S, C):
    nc = tc.nc
    CK = C // P      # context seq chunks (16)
    sqb = min(SQB, S)  # phase-2 query block

    xt_r = xt.rearrange("(ko p) s -> p ko s", p=P)
    ctxt_r = ctxt.rearrange("(ko p) s -> p ko s", p=P)
    out_r = out_t.rearrange("(ko p) s -> ko p s", p=P)

    with (
        tc.tile_pool(name="w", bufs=1) as wpool,
        tc.tile_pool(name="wbig", bufs=2) as wbig,
        tc.tile_pool(name="acts", bufs=2) as actsp,
        tc.tile_pool(name="kqv", bufs=1) as kqv,
        tc.tile_pool(name="lora", bufs=1) as lorap,
        tc.tile_pool(name="pt", bufs=4) as ptp,
        tc.tile_pool(name="small", bufs=2) as smallp,
        tc.tile_pool(name="outsb", bufs=2) as outp,
    ):
        # ---- small weights (resident) ----
        aq_sb = wpool.tile([P, KO, R], BF16, tag="aq")
        nc.sync.dma_start(aq_sb[:], aqT.rearrange("(ko p) r -> p ko r", p=P))
        av_sb = wpool.tile([P, KO, R], BF16, tag="av")
        nc.sync.dma_start(av_sb[:], avT.rearrange("(ko p) r -> p ko r", p=P))
        bq_sb = wpool.tile([R, GD], BF16, tag="bq")
        nc.sync.dma_start(bq_sb[:], bqT)
        bv_sb = wpool.tile([R, GD], BF16, tag="bv")
        nc.sync.dma_start(bv_sb[:], bvT)

        # ---- big weights cycle through 2 slots: wk, wv, wq, wo ----
        wk_sb = wbig.tile([P, KO, GD], BF16, tag="wbig")
        nc.sync.dma_start(wk_sb[:], wkT.rearrange("(ko p) m -> p ko m", p=P))
        wv_sb = wbig.tile([P, KO, GD], BF16, tag="wbig")
        nc.sync.dma_start(wv_sb[:], wvT.rearrange("(ko p) m -> p ko m", p=P))

        # ---- persistent activations ----
        # kt_z / vaug_z are zero-padded so every phase-2 matmul drives the
        # FULL 128x128 PE array (half-array matmuls keep the HAM clock gate
        # cold at 1.2 GHz -- measured 427 ns/MM instead of 213 ns).
        # kt_z[:, h]: rows (h%2)*64..+64 hold K_h^T, other 64 rows are zero.
        # vaug_z[:, sk, h]: cols 0..63 = V_h, col 64 = ones, cols 65..127 = 0.
        kt_z = kqv.tile([P, HG, C], BF16, tag="kt")       # K^T  [hd, ctx]
        qt_sb = kqv.tile([P, 2, S], BF16, tag="qt")       # Q^T  [hd, seq]
        vaug_z = kqv.tile([P, CK, HG, P], BF16, tag="vaug")
        att_sb = kqv.tile([P, 2, S], BF16, tag="att")     # attn out^T (normalized)
        tv_sb = lorap.tile([R, C], BF16, tag="tv")
        tq_sb = lorap.tile([R, S], BF16, tag="tq")

        nc.vector.memset(kt_z[:], 0.0)
        nc.vector.memset(vaug_z[:], 0.0)
        nc.vector.memset(vaug_z[:, :, :, HD], 1.0)

        # ================= phase 1a: context -> Kt, V =================
        with (
            tc.tile_pool(name="psum1", bufs=4, space="PSUM") as psum1,
            tc.tile_pool(name="psumv", bufs=2, space="PSUM") as psumv,
        ):
            for q in range(C // AQ):
                sl = slice(q * AQ, (q + 1) * AQ)
                ctx_sb = actsp.tile([P, KO, AQ], BF16, tag="acts")
                nc.sync.dma_start(ctx_sb[:], ctxt_r[:, :, sl])

                # tv = Av @ ctx^T  -> [R, ctx-quarter]
                tvp = psum1.tile([P, NMM], F32, tag="proj")
                for k in range(KO):
                    nc.tensor.matmul(
                        tvp[:], (av_sb[:, k, :]), (ctx_sb[:, k, :]),
                        start=(k == 0), stop=(k == KO - 1),
                    )
                nc.vector.tensor_copy(tv_sb[:, sl], tvp[:])

                # Kt quarter (rows 0:64 -> head 2m, rows 64:128 -> head 2m+1)
                for m in range(2):
                    kp = psum1.tile([P, NMM], F32, tag="proj")
                    for k in range(KO):
                        nc.tensor.matmul(
                            kp[:], (wk_sb[:, k, ts(m, P)]), (ctx_sb[:, k, :]),
                            start=(k == 0), stop=(k == KO - 1),
                        )
                    nc.vector.tensor_copy(kt_z[0:HD, 2 * m, sl], kp[0:HD, :])
                    nc.vector.tensor_copy(kt_z[HD:P, 2 * m + 1, sl], kp[HD:P, :])

                # V quarter (normal layout, head-interleaved with ones col)
                for mloc in range(AQ // P):
                    vp = psumv.tile([P, GD], F32, tag="vproj")
                    for k in range(KO):
                        nc.tensor.matmul(
                            vp[:], (ctx_sb[:, k, ts(mloc, P)]), (wv_sb[:, k, :]),
                            start=(k == 0), stop=False,
                        )
                    nc.tensor.matmul(
                        vp[:], (tv_sb[:, q * AQ + mloc * P:q * AQ + (mloc + 1) * P]),
                        (bv_sb[:]), start=False, stop=True,
                    )
                    mg = q * (AQ // P) + mloc
                    nc.vector.tensor_copy(
                        vaug_z[:, mg, :, 0:HD],
                        vp[:].rearrange("p (h d) -> p h d", h=HG),
                    )

            # ================= phase 1b: x -> Qt =================
            wq_sb = wbig.tile([P, KO, GD], BF16, tag="wbig")
            nc.sync.dma_start(wq_sb[:], wqT.rearrange("(ko p) m -> p ko m", p=P))

            for q in range(S // AQ):
                sl = slice(q * AQ, (q + 1) * AQ)
                x_sb = actsp.tile([P, KO, AQ], BF16, tag="acts")
                nc.sync.dma_start(x_sb[:], xt_r[:, :, sl])

                tqp = psum1.tile([P, NMM], F32, tag="proj")
                for k in range(KO):
                    nc.tensor.matmul(
                        tqp[:], (aq_sb[:, k, :]), (x_sb[:, k, :]),
                        start=(k == 0), stop=(k == KO - 1),
                    )
                nc.vector.tensor_copy(tq_sb[:, sl], tqp[:])

                for m in range(2):
                    qp = psum1.tile([P, NMM], F32, tag="proj")
                    for k in range(KO):
                        nc.tensor.matmul(
                            qp[:], (wq_sb[:, k, ts(m, P)]), (x_sb[:, k, :]),
                            start=(k == 0), stop=False,
                        )
                    nc.tensor.matmul(
                        qp[:], (bq_sb[:, ts(m, P)]), (tq_sb[:, sl]),
                        start=False, stop=True,
                    )
                    nc.vector.tensor_copy(qt_sb[:, m, sl], qp[:])

        # ================= phase 2: attention =================
        wo_sb = wbig.tile([P, 2, D], BF16, tag="wbig")
        nc.sync.dma_start(wo_sb[:], woT.rearrange("(j p) d -> p j d", p=P))

        with (
            tc.tile_pool(name="st", bufs=2, space="PSUM") as stp,
            tc.tile_pool(name="ot", bufs=2, space="PSUM") as otp,
        ):
            for qb in range(S // sqb):
                for h in range(HG):
                    hp = (h % 2) * HD
                    hc = h // 2
                    ot = otp.tile([P, sqb], F32, tag="ot")

                    def attn_v(sk, pt):
                        for n in range(sqb // NMM):
                            nc.tensor.matmul(
                                ot[:, ts(n, NMM)],
                                (vaug_z[:, sk, h, :]),
                                (pt[:, ts(n, NMM)]),
                                start=(sk == 0), stop=(sk == CK - 1),
                            )

                    # software-pipelined: attnV for iteration sk-1 is emitted
                    # after scores/exp of iteration sk, so the PE stream never
                    # head-of-line blocks on the current iteration's ACT.
                    prev = None
                    for sk in range(CK):
                        st = stp.tile([P, sqb], F32, tag="st")
                        for n in range(sqb // NMM):
                            nc.tensor.matmul(
                                st[:, ts(n, NMM)],
                                (kt_z[:, h, ts(sk, P)]),
                                (qt_sb[:, hc,
                                       qb * sqb + n * NMM:qb * sqb + (n + 1) * NMM]),
                                start=True, stop=True,
                            )
                        pt = ptp.tile([P, sqb], BF16, tag="pt")
                        nc.scalar.activation(pt[:], st[:], EXP)
                        if prev is not None:
                            attn_v(*prev)
                        prev = (sk, pt)
                    attn_v(*prev)
                    # normalize: rows 0..63 are O^T, row 64 is the exp rowsum
                    rr = smallp.tile([1, sqb], F32, tag="rr")
                    nc.vector.tensor_copy(rr[:], ot[HD:HD + 1, :])
                    rf = smallp.tile([1, sqb], F32, tag="rf")
                    nc.vector.reciprocal_approx_fast(rf[:], rr[:])
                    rb = smallp.tile([HD, sqb], F32, tag="rb")
                    nc.gpsimd.partition_broadcast(rb[:], rf[:])
                    nc.vector.tensor_mul(
                        att_sb[hp:hp + HD, hc,
                               qb * sqb:(qb + 1) * sqb],
                        ot[0:HD, :], rb[:],
                    )

                # ---- out-projection for this query block (PSUM via st tag) ----
                for e in range(KO):
                    osb = outp.tile([P, sqb], F32, tag="osb")
                    for n in range(sqb // NMM):
                        ng = qb * (sqb // NMM) + n
                        op = stp.tile([P, NMM], F32, tag="st")
                        for j in range(2):
                            nc.tensor.matmul(
                                op[:], (wo_sb[:, j, ts(e, P)]),
                                (att_sb[:, j, ts(ng, NMM)]),
                                start=(j == 0), stop=(j == 1),
                            )
                        nc.vector.tensor_copy(osb[:, ts(n, NMM)], op[:])
                    nc.sync.dma_start(out_r[e][:, qb * sqb:(qb + 1) * sqb], osb[:])


# ---------------------------------------------------------------------------
# Host side
# ---------------------------------------------------------------------------

_NC_CACHE = {}


def _get_nc(S=2048, C=2048):
    key = (S, C)
    if key not in _NC_CACHE:
        _NC_CACHE[key] = build_nc(S, C)
    return _NC_CACHE[key]


def shard_inputs(x, context, Wq, Aq, Bq, Wk, Wv, Av, Bv, Wo):
    """Build the 8 per-core input maps (host-side shard + transpose + scale +
    bf16 cast)."""
    import ml_dtypes

    bf16 = ml_dtypes.bfloat16
    f = lambda a: np.ascontiguousarray(np.asarray(a, dtype=np.float32))
    c = lambda a: np.ascontiguousarray(a).astype(bf16)
    x, context = f(x), f(context)
    Wq, Aq, Bq, Wk, Wv, Av, Bv, Wo = map(f, (Wq, Aq, Bq, Wk, Wv, Av, Bv, Wo))
    sd = 8.0  # sqrt(head_dim)
    lr = 128.0  # LoRA rank (scale = 1/r)
    aqT = c(Aq.T)
    avT = c(Av.T)
    in_maps = []
    for core in range(8):
        b, g = core // 4, core % 4
        sl = slice(g * GD, (g + 1) * GD)
        in_maps.append({
            "xt": c(x[b].T),
            "ctxt": c(context[b].T),
            "wqT": c(Wq[sl].T / sd),
            "aqT": aqT,
            "bqT": c(Bq[sl].T / (lr * sd)),
            "wkT": c(Wk[sl].T),
            "wvT": c(Wv[sl].T),
            "avT": avT,
            "bvT": c(Bv[sl].T / lr),
            "woT": c(Wo[:, sl].T),
        })
    return in_maps


def unshard_output(results, B=2, S=2048):
    out = np.zeros((B, S, D), np.float32)
    for core in range(8):
        b = core // 4
        out[b] += results[core]["out_t"].T
    return out


def kernel(x, context, Wq, Aq, Bq, Wk, Wv, Av, Bv, Wo, _trace=False):
    nc = _get_nc()
    in_maps = shard_inputs(x, context, Wq, Aq, Bq, Wk, Wv, Av, Bv, Wo)
    res = run_bass_kernel_spmd(nc, in_maps, core_ids=list(range(8)), trace=_trace)
    out = unshard_output(res.results)
    if _trace:
        kernel.last_result = res
    return out
